# revision 42
# baseline (speedup 1.0000x reference)
"""AngularLoss Trainium2 kernel (8 NeuronCores, SPMD data-parallel).

Computation (reference):
    t2  = tan(alpha_deg * pi/180)^2
    apn = rowsum((a + p) * n)          # [N,1]
    ap  = rowsum(a * p)                # [N,1]
    f   = 4*t2*apn - 2*(1+t2)*ap       # [N,1]
    out = logsumexp(f, axis=0)         # [1]

Strategy: shard N=262144 rows across 8 cores (32768 rows each).  Each core
streams its 3x16MB f32 shard through SBUF.  All three tensors load f32 on
the single HWDGE (sync) ring: one FIFO keeps every SDMA engine at line
rate with no SWDGE descriptor-ring contention (which throttles engines
7/15) and no cross-ring round-robin imbalance.  The stream runs at the
chip HBM ceiling (~320 GB/s per core with all 8 cores pulling), so the
remaining schedule minimizes ramp and drain: chunk sizes taper at the end
(each chunk's DVE time fits inside the next chunk's stream time) so the
vector engine never builds a backlog and the post-stream drain is just
the last small chunk.  ScalarE casts each tile to bf16 so every DVE
tensor_tensor runs in 2x bf16 mode; per-row dots fold (depth tuned per
chunk size) then grouped X-reduce.  f-values reduce to per-partition
(max, sum-exp) partials in column groups that close at chunk boundaries;
group logsumexp tails run on ScalarE (negm, exp, store on the scalar
HWDGE ring) so the final chain has one cross-engine hop.  Output per core
is [128, 2*n_groups] = (m,s) per partition per group; the host combines
all partials into the final logsumexp (permutation-invariant, so the
flat DRAM tiling needs no row bookkeeping).
"""

import numpy as np

import concourse.bacc as bacc
import concourse.bass as bass
import concourse.tile as tile
from concourse import mybir
from concourse.bass_utils import run_bass_kernel_spmd

N, D = 262144, 128
NCORES = 8
N_LOCAL = N // NCORES            # 32768 rows per core
P = 128                          # partitions
ROWS_PER_PART = N_LOCAL // P     # 256 rows owned by each partition
F32 = mybir.dt.float32
BF16 = mybir.dt.bfloat16

# row-chunk schedule (rows per partition per DMA load): big 2MB loads in
# steady state, tapering so chunk k's DVE work fits in chunk k+1's
# stream time (no compute backlog at the end of the stream).  The taper
# decays toward B*~7 where DVE(B) equals the arrival spacing -- chunks
# below that accumulate backlog again (fixed per-chunk DVE overhead).
CHUNKS = [32] * 4 + [25, 21, 18, 16, 14, 13, 11, 10]
assert sum(CHUNKS) == ROWS_PER_PART
# logsumexp group boundaries (cols); must land on chunk boundaries
GROUP_BOUNDS = [64, 128, 174, 222, 246, 256]
N_GROUPS = len(GROUP_BOUNDS)
_cum = np.cumsum(CHUNKS).tolist()
assert all(b in _cum for b in GROUP_BOUNDS)


def _n_folds(B: int) -> int:
    # fold (bf16 TT 2x) halves the 1x reduce work; each fold costs a
    # fixed ~151-cycle issue, so shallow folds win for small chunks.
    if B >= 19:
        return 3
    if B >= 10:
        return 2
    if B >= 5:
        return 1
    return 0


def _build(c1: float, c2: float) -> bass.Bass:
    k12 = c1 / c2  # f = c2 * rowsum(k12*t + u), t=(a+p)*n, u=a*p
    nc = bacc.Bacc()
    a_ext = nc.declare_dram_parameter("anchor", [N_LOCAL, D], F32, isOutput=False)
    p_ext = nc.declare_dram_parameter("positive", [N_LOCAL, D], F32, isOutput=False)
    n_ext = nc.declare_dram_parameter("negative", [N_LOCAL, D], F32, isOutput=False)
    out_ext = nc.declare_dram_parameter("out", [P, 2 * N_GROUPS], F32, isOutput=True)

    group_lo = [0] + GROUP_BOUNDS[:-1]
    group_sz = [hi - lo for lo, hi in zip(group_lo, GROUP_BOUNDS)]

    with tile.TileContext(nc) as tc:
        with (
            tc.tile_pool(name="ina", bufs=2) as pool_a,
            tc.tile_pool(name="inp", bufs=2) as pool_p,
            tc.tile_pool(name="inn", bufs=3) as pool_n,
            tc.tile_pool(name="abf", bufs=2) as pool_abf,
            tc.tile_pool(name="pbf", bufs=2) as pool_pbf,
            tc.tile_pool(name="nbf", bufs=3) as pool_nbf,
            tc.tile_pool(name="work", bufs=1) as pool_w,
            tc.tile_pool(name="fold", bufs=1) as pool_f,
            tc.tile_pool(name="acc", bufs=1) as pool_acc,
        ):
            s_g = [
                pool_acc.tile([P, group_sz[g]], F32, name=f"s{g}", tag=f"s{g}")
                for g in range(N_GROUPS)
            ]
            f_g = [
                pool_acc.tile([P, group_sz[g]], F32, name=f"f{g}", tag=f"f{g}")
                for g in range(N_GROUPS)
            ]
            expf_g = [
                pool_acc.tile([P, group_sz[g]], F32, name=f"expf{g}", tag=f"expf{g}")
                for g in range(N_GROUPS)
            ]
            negm_g = [
                pool_acc.tile([P, 1], F32, name=f"negm{g}", tag=f"negm{g}")
                for g in range(N_GROUPS)
            ]
            # one (m, s) tile for all groups -> single batched store at end
            ms_all = pool_acc.tile([P, 2 * N_GROUPS], F32, name="ms", tag="ms")
            ms_g = [ms_all[:, 2 * g : 2 * g + 2] for g in range(N_GROUPS)]

            def lse_vec(g):
                # DVE part of group g's logsumexp: f = c2*s, max.  Runs at
                # the group boundary -- inputs are complete, so it never
                # stalls the DVE queue.
                nc.vector.tensor_scalar_mul(f_g[g][:], s_g[g][:], c2)
                nc.vector.tensor_reduce(
                    out=ms_all[:, 2 * g : 2 * g + 1], in_=f_g[g][:],
                    axis=mybir.AxisListType.X, op=mybir.AluOpType.max,
                )

            def lse_act(g):
                # ScalarE part (negm, exp+accum).  Deferred 2+ chunks past
                # the boundary: EXP waits on the DVE max semaphore, and the
                # in-order ACT queue would otherwise head-block the next
                # chunks' casts behind that wait.
                nc.scalar.activation(
                    out=negm_g[g][:], in_=ms_all[:, 2 * g : 2 * g + 1],
                    func=mybir.ActivationFunctionType.Copy, scale=-1.0,
                )
                nc.scalar.activation(
                    out=expf_g[g][:], in_=f_g[g][:],
                    func=mybir.ActivationFunctionType.Exp,
                    bias=negm_g[g][:], scale=1.0,
                    accum_out=ms_all[:, 2 * g + 1 : 2 * g + 2],
                )

            col = 0
            group_done = 0
            pending_act = []  # (group, chunk index when its lse_vec ran)
            for ci, B in enumerate(CHUNKS):
                # flat tiling: chunk covers global rows [128*col, 128*(col+B));
                # partition p gets B consecutive rows -- fully contiguous span.
                r0 = P * col
                va = a_ext[r0 : r0 + P * B, :].rearrange("(p r) d -> p r d", p=P)
                vp = p_ext[r0 : r0 + P * B, :].rearrange("(p r) d -> p r d", p=P)
                vn = n_ext[r0 : r0 + P * B, :].rearrange("(p r) d -> p r d", p=P)

                ta32 = pool_a.tile([P, B * D], F32, tag="ina")
                tp32 = pool_p.tile([P, B * D], F32, tag="inp")
                tn32 = pool_n.tile([P, B * D], F32, tag="inn")
                nc.sync.dma_start(out=ta32[:], in_=va)
                nc.sync.dma_start(out=tp32[:], in_=vp)
                nc.sync.dma_start(out=tn32[:], in_=vn)

                # ScalarE casts keep every DVE op in 2x bf16 mode.  The n
                # cast also applies the k12 = c1/c2 scale, so the two dot
                # branches merge later with a plain 2x TT add.
                tabf = pool_abf.tile([P, B * D], BF16, tag="abf")
                tpbf = pool_pbf.tile([P, B * D], BF16, tag="pbf")
                tnbf = pool_nbf.tile([P, B * D], BF16, tag="nbf")
                for dst, src, sc in (
                    (tabf, ta32, 1.0),
                    (tpbf, tp32, 1.0),
                    (tnbf, tn32, k12),
                ):
                    nc.scalar.activation(
                        out=dst[:], in_=src[:],
                        func=mybir.ActivationFunctionType.Copy, scale=sc,
                    )

                tq = pool_w.tile([P, B * D], BF16, tag="q")
                tt = pool_w.tile([P, B * D], BF16, tag="t")
                tu = pool_w.tile([P, B * D], BF16, tag="u")
                nc.vector.tensor_tensor(tq[:], tabf[:], tpbf[:], mybir.AluOpType.add)
                nc.vector.tensor_tensor(tt[:], tq[:], tnbf[:], mybir.AluOpType.mult)
                nc.vector.tensor_tensor(tu[:], tabf[:], tpbf[:], mybir.AluOpType.mult)

                # Merge branches: t already carries k12 (via the n cast), so
                # w = t + u folds both dots into one chain at 2x.  w reuses
                # q's buffer (q's last reader tt precedes the write).
                tw = pool_w.tile([P, B * D], BF16, tag="q")
                nc.vector.tensor_tensor(tw[:], tt[:], tu[:], mybir.AluOpType.add)
                # Per-row dots: fold halves in bf16 2x (depth tuned per B),
                # then grouped X-reduce (1x) into accumulator columns.
                g = group_done
                gcol = col - group_lo[g]
                assert gcol + B <= group_sz[g]
                cur = tw
                w = D
                for lvl in range(_n_folds(B)):
                    v3 = cur[:].rearrange("p (r d) -> p r d", d=w)
                    tf = pool_f.tile([P, B * (w // 2)], BF16, tag=f"f{lvl + 1}")
                    nc.vector.tensor_tensor(
                        tf[:], v3[:, :, : w // 2], v3[:, :, w // 2 :],
                        mybir.AluOpType.add,
                    )
                    cur = tf
                    w //= 2
                nc.vector.tensor_reduce(
                    out=s_g[g][:, gcol : gcol + B],
                    in_=cur[:].rearrange("p (r d) -> p r d", d=w),
                    axis=mybir.AxisListType.X,
                    op=mybir.AluOpType.add,
                )
                col += B
                while group_done < N_GROUPS and col >= GROUP_BOUNDS[group_done]:
                    lse_vec(group_done)
                    pending_act.append((group_done, ci))
                    group_done += 1
                # Flush deferred ACT tails only while well before the end:
                # a flushed negm/EXP waits on its group's DVE max, and in
                # the tail (DVE backlogged) that wait would head-block the
                # final chunks' casts queued behind it.
                while (
                    pending_act
                    and ci >= pending_act[0][1] + 2
                    and ci <= len(CHUNKS) - 4
                ):
                    lse_act(pending_act.pop(0)[0])
            for g, _ in pending_act:
                lse_act(g)
            # single batched store of every group's (m, s)
            nc.scalar.dma_start(out=out_ext[:], in_=ms_all[:])
    nc.compile()
    return nc


def kernel(anchor, positive, negative, alpha):
    anchor = np.ascontiguousarray(np.asarray(anchor, dtype=np.float32))
    positive = np.ascontiguousarray(np.asarray(positive, dtype=np.float32))
    negative = np.ascontiguousarray(np.asarray(negative, dtype=np.float32))
    a_rad = 2.0 * np.pi * float(np.asarray(alpha)) / 360.0
    t2 = float(np.tan(a_rad) ** 2)
    c1 = 4.0 * t2
    c2 = -2.0 * (1.0 + t2)

    nc = _build(c1, c2)
    in_maps = []
    for i in range(NCORES):
        sl = slice(i * N_LOCAL, (i + 1) * N_LOCAL)
        in_maps.append(
            {"anchor": anchor[sl], "positive": positive[sl], "negative": negative[sl]}
        )
    res = run_bass_kernel_spmd(nc, in_maps, core_ids=list(range(NCORES)))

    ms = np.concatenate([np.asarray(r["out"]) for r in res.results], axis=0)
    m = ms[:, 0::2].reshape(-1).astype(np.float64)
    s = ms[:, 1::2].reshape(-1).astype(np.float64)
    M = m.max()
    S = np.sum(s * np.exp(m - M))
    return np.array([np.log(S) + M], dtype=np.float32)


if __name__ == "__main__":
    rng = np.random.default_rng(0)
    out = kernel(
        anchor=rng.standard_normal((N, D), dtype=np.float32),
        positive=rng.standard_normal((N, D), dtype=np.float32),
        negative=rng.standard_normal((N, D), dtype=np.float32),
        alpha=np.int64(45),
    )
    print("kernel out:", out)
